# revision 5
# baseline (speedup 1.0000x reference)
"""Trainium2 Bass kernel for nn_EnhancedMemoryStack (scatter_memory).

Strategy (pure data parallel, batch sharded 8 ways):
  Per core: 4096 batches. Rows r=(bi,s) b-major, tile = 8 batches = 128 rows.
  - Fold the six 64x64 complex projections into 128x128 real mats on host
    (weight constant-folding): Wq,Wk,Wv; and A = Wq @ Wk.T * D^-0.5 so that
    scores = X A X^T needs ONE projection (Y = X @ A) instead of Q and K.
  - mem_new = mem*(1-push) + push*z via PE partition-broadcast of z*push
    (onesbd matmul) + fused scalar_tensor_tensor.
  - Per 128-row tile: XT = PE-transpose(mem_new); YT = A.T @ XT;
    V = X @ Wv (row layout); scores = Y X^T (psum 128x128, 8 batches
    block-diagonal); E = exp(scores)*mask (block mask) with fused rowsum;
    w = E.T @ (new_ptr/rowsum); z_read^T = Vw.T @ G.
  - new_ptr/gates computed once per core in a wide column layout; ptr inputs
    are host-permuted (pure layout transforms) so all DMA is contiguous.
"""

import numpy as np

import concourse.bass as bass
import concourse.tile as tile
from concourse import bacc, mybir
from concourse import bass_utils

B, S, D = 32768, 16, 64
F = 2 * D            # 128
NCORES = 8
BC = B // NCORES     # 4096 batches per core
ROWS = BC * S        # 65536 rows per core
T = BC // 8          # 512 tiles per core
C = 4                # tiles per chunk
NCH = T // C         # chunks per core
NQ = BC // 128       # 32 column-groups in wide layouts
EPS = 1e-6

f32 = mybir.dt.float32
AF = mybir.ActivationFunctionType
ALU = mybir.AluOpType

_CACHE = {}


def _build_program():
    nc = bacc.Bacc(
        "TRN2",
        target_bir_lowering=False,
        debug=False,
        enable_asserts=False,
        num_devices=NCORES,
    )

    def din(name, shape):
        return nc.dram_tensor(name, list(shape), f32, kind="ExternalInput").ap()

    def dout(name, shape):
        return nc.dram_tensor(name, list(shape), f32, kind="ExternalOutput").ap()

    mem_in = din("mem_in", (ROWS, F))
    z_in = din("z_in", (128, NQ * F))        # z_wide[p, q*F+f] = zflat[q*128+p, f]
    ctrlw_in = din("ctrlw_in", (128, NQ * 3))  # wide: [p, q*3+j] = ctrl[q*128+p, j]
    ctrlT_in = din("ctrlT_in", (128, T * 3))   # col:  [bi*16+s, t*3+j] = ctrl[t*8+bi, j]
    ptrv_in = din("ptrv_in", (128, T))         # col:  [bi*16+s, t] = ptr[t*8+bi, s]
    ptru_in = din("ptru_in", (128, T))         # same, ptr rolled +1 along s
    ptrd_in = din("ptrd_in", (128, T))         # same, ptr rolled -1 along s
    ident_in = din("ident_in", (128, 128))
    a_in = din("a_in", (128, 128))
    wv_in = din("wv_in", (128, 128))
    ones8_in = din("ones8_in", (8, 128))       # kron(eye(8), ones(1,16))
    mask_in = din("mask_in", (128, 128))       # kron(eye(8), ones(16,16))
    g_in = din("g_in", (128, 8))               # kron(eye(8), ones(16,1))

    out_mem = dout("out_mem", (ROWS, F))
    out_nptr = dout("out_nptr", (128, T))
    out_zr = dout("out_zr", (BC, F))
    out_cnt = dout("out_cnt", (128, 1))

    with tile.TileContext(nc) as tc:
        with (
            tc.tile_pool(name="const", bufs=1) as cpool,
            tc.tile_pool(name="sb", bufs=3) as sb,
            tc.tile_pool(name="sb2", bufs=3) as sb2,
            tc.tile_pool(name="ps", bufs=2, space=bass.MemorySpace.PSUM) as ps,
            tc.tile_pool(name="psb", bufs=1, space=bass.MemorySpace.PSUM) as psb,
        ):
            # ---- resident constants ----
            ident = cpool.tile([128, 128], f32)
            nc.sync.dma_start(ident[:], ident_in)
            a_t = cpool.tile([128, 128], f32)
            nc.sync.dma_start(a_t[:], a_in)
            wv_t = cpool.tile([128, 128], f32)
            nc.sync.dma_start(wv_t[:], wv_in)
            ones8 = cpool.tile([8, 128], f32)
            nc.sync.dma_start(ones8[:], ones8_in)
            mask_t = cpool.tile([128, 128], f32)
            nc.sync.dma_start(mask_t[:], mask_in)
            g_t = cpool.tile([128, 8], f32)
            nc.sync.dma_start(g_t[:], g_in)

            ctrlT = cpool.tile([128, T * 3], f32)
            nc.sync.dma_start(ctrlT[:], ctrlT_in)
            ptrv = cpool.tile([128, T], f32)
            nc.sync.dma_start(ptrv[:], ptrv_in)
            ptru = cpool.tile([128, T], f32)
            nc.sync.dma_start(ptru[:], ptru_in)
            ptrd = cpool.tile([128, T], f32)
            nc.sync.dma_start(ptrd[:], ptrd_in)
            zw = cpool.tile([128, NQ * F], f32)
            nc.sync.dma_start(zw[:], z_in)
            ctrlw = cpool.tile([128, NQ * 3], f32)
            nc.sync.dma_start(ctrlw[:], ctrlw_in)

            # ---- gates in column layout (once per core) ----
            g3 = cpool.tile([128, T * 3], f32)
            nc.scalar.activation(g3[:], ctrlT[:], AF.Sigmoid)
            g3v = g3[:].rearrange("p (t j) -> p t j", j=3)
            tot = cpool.tile([128, T], f32)
            nc.vector.tensor_reduce(tot[:], g3v, axis=mybir.AxisListType.X, op=ALU.add)
            nc.vector.tensor_scalar_add(tot[:], tot[:], EPS)
            rec = cpool.tile([128, T], f32)
            nc.vector.reciprocal(rec[:], tot[:])
            push = cpool.tile([128, T], f32)
            nc.vector.tensor_mul(push[:], g3v[:, :, 0], rec[:])
            omp = cpool.tile([128, T], f32)   # 1 - push
            nc.vector.tensor_scalar(omp[:], push[:], -1.0, 1.0, op0=ALU.mult, op1=ALU.add)
            npt = cpool.tile([128, T], f32)   # new_ptr (column layout)
            tmp = cpool.tile([128, T], f32)
            # new_ptr = push*up + pop*down + stay*ptr
            nc.vector.tensor_mul(npt[:], push[:], ptru[:])
            nc.vector.tensor_mul(tmp[:], g3v[:, :, 1], rec[:])       # pop
            nc.vector.tensor_mul(tmp[:], tmp[:], ptrd[:])
            nc.vector.tensor_add(npt[:], npt[:], tmp[:])
            nc.vector.tensor_mul(tmp[:], g3v[:, :, 2], rec[:])       # stay
            nc.vector.tensor_mul(tmp[:], tmp[:], ptrv[:])
            nc.vector.tensor_add(npt[:], npt[:], tmp[:])
            nc.sync.dma_start(out_nptr, npt[:])
            # active slot count partials
            gt = cpool.tile([128, T], f32)
            nc.vector.tensor_scalar(gt[:], npt[:], 0.1, None, op0=ALU.is_gt)
            cnt = cpool.tile([128, 1], f32)
            nc.vector.tensor_reduce(cnt[:], gt[:], axis=mybir.AxisListType.X, op=ALU.add)
            nc.sync.dma_start(out_cnt, cnt[:])

            # ---- zp_wide = z * push (wide layout, once per core) ----
            g3w = cpool.tile([128, NQ * 3], f32)
            nc.scalar.activation(g3w[:], ctrlw[:], AF.Sigmoid)
            g3wv = g3w[:].rearrange("p (q j) -> p q j", j=3)
            totw = cpool.tile([128, NQ], f32)
            nc.vector.tensor_reduce(totw[:], g3wv, axis=mybir.AxisListType.X, op=ALU.add)
            nc.vector.tensor_scalar_add(totw[:], totw[:], EPS)
            recw = cpool.tile([128, NQ], f32)
            nc.vector.reciprocal(recw[:], totw[:])
            pushw = cpool.tile([128, NQ], f32)
            nc.vector.tensor_mul(pushw[:], g3wv[:, :, 0], recw[:])
            zp = cpool.tile([128, NQ * F], f32)
            pushw_b = pushw[:].unsqueeze(-1).broadcast_to([128, NQ, F])
            nc.vector.tensor_mul(
                zp[:].rearrange("p (q f) -> p q f", f=F), zw[:].rearrange("p (q f) -> p q f", f=F), pushw_b
            )

            # ---- z_read accumulator (transposed layout) ----
            rt_all = cpool.tile([128, T * 8], f32)

            # ---- main loop over chunks ----
            for ch in range(NCH):
                t0 = ch * C
                r0 = t0 * 128
                CF = C * F

                mem_c = sb.tile([128, CF], f32, tag="mem")
                nc.sync.dma_start(
                    mem_c[:].rearrange("p (c f) -> p c f", f=F),
                    mem_in[r0 : r0 + C * 128, :].rearrange("(c p) f -> p c f", p=128),
                )
                # stage zp slices for this chunk into partitions 0..7
                zp8 = sb.tile([8, CF], f32, tag="zp8")
                for c in range(C):
                    po = ((t0 + c) * 8) % 128
                    q = ((t0 + c) * 8) // 128
                    nc.sync.dma_start(
                        zp8[:, c * F : (c + 1) * F],
                        zp[po : po + 8, q * F : (q + 1) * F],
                    )
                psum_z = psb.tile([128, CF], f32, tag="pz")
                for c in range(C):
                    nc.tensor.matmul(
                        psum_z[:, c * F : (c + 1) * F], ones8[:], zp8[:, c * F : (c + 1) * F],
                        start=True, stop=True,
                    )
                mnew = sb.tile([128, CF], f32, tag="mnew")
                for c in range(C):
                    cs = slice(c * F, (c + 1) * F)
                    nc.vector.scalar_tensor_tensor(
                        mnew[:, cs], mem_c[:, cs], omp[:, t0 + c : t0 + c + 1], psum_z[:, cs],
                        op0=ALU.mult, op1=ALU.add,
                    )
                nc.sync.dma_start(
                    out_mem[r0 : r0 + C * 128, :].rearrange("(c p) f -> p c f", p=128),
                    mnew[:].rearrange("p (c f) -> p c f", f=F),
                )
                psum_xt = psb.tile([128, CF], f32, tag="pxt")
                for c in range(C):
                    cs = slice(c * F, (c + 1) * F)
                    nc.tensor.transpose(psum_xt[:, cs], mnew[:, cs], ident[:])
                xt = sb.tile([128, CF], f32, tag="xt")
                nc.scalar.activation(xt[:], psum_xt[:], AF.Copy)
                psum_yt = psb.tile([128, CF], f32, tag="pyt")
                for c in range(C):
                    cs = slice(c * F, (c + 1) * F)
                    nc.tensor.matmul(psum_yt[:, cs], a_t[:], xt[:, cs], start=True, stop=True)
                yt = sb2.tile([128, CF], f32, tag="yt")
                nc.scalar.activation(yt[:], psum_yt[:], AF.Copy)
                psum_v = psb.tile([128, CF], f32, tag="pv")
                for c in range(C):
                    cs = slice(c * F, (c + 1) * F)
                    nc.tensor.matmul(psum_v[:, cs], xt[:, cs], wv_t[:], start=True, stop=True)
                psum_sc = ps.tile([128, CF], f32, tag="psc")
                for c in range(C):
                    cs = slice(c * F, (c + 1) * F)
                    nc.tensor.matmul(psum_sc[:, cs], yt[:, cs], xt[:, cs], start=True, stop=True)
                eraw = sb2.tile([128, CF], f32, tag="eraw")
                nc.scalar.activation(eraw[:], psum_sc[:], AF.Exp)
                e_t = sb2.tile([128, CF], f32, tag="e")
                rs = sb.tile([128, C], f32, tag="rs")
                for c in range(C):
                    cs = slice(c * F, (c + 1) * F)
                    nc.vector.scalar_tensor_tensor(
                        e_t[:, cs], eraw[:, cs], 1.0, mask_t[:],
                        op0=ALU.mult, op1=ALU.mult,
                        accum_out=rs[:, c : c + 1],
                    )
                rr = sb.tile([128, C], f32, tag="rr")
                nc.vector.reciprocal(rr[:], rs[:])
                ptrn = sb.tile([128, C], f32, tag="ptrn")
                nc.vector.tensor_mul(ptrn[:], npt[:, t0 : t0 + C], rr[:])
                psum_wr = psb.tile([128, C + C * 8], f32, tag="pwr")
                for c in range(C):
                    cs = slice(c * F, (c + 1) * F)
                    nc.tensor.matmul(
                        psum_wr[:, c : c + 1], e_t[:, cs], ptrn[:, c : c + 1],
                        start=True, stop=True,
                    )
                wsb = sb.tile([128, C], f32, tag="wsb")
                nc.vector.tensor_copy(wsb[:], psum_wr[:, 0:C])
                vw = sb2.tile([128, CF], f32, tag="vw")
                wsb_b = wsb[:].unsqueeze(-1).broadcast_to([128, C, F])
                nc.vector.tensor_mul(
                    vw[:].rearrange("p (c f) -> p c f", f=F),
                    psum_v[:].rearrange("p (c f) -> p c f", f=F),
                    wsb_b,
                )
                for c in range(C):
                    cs = slice(c * F, (c + 1) * F)
                    nc.tensor.matmul(
                        psum_wr[:, C + c * 8 : C + (c + 1) * 8], vw[:, cs], g_t[:],
                        start=True, stop=True,
                    )
                nc.vector.tensor_copy(
                    rt_all[:, ch * C * 8 : (ch + 1) * C * 8], psum_wr[:, C : C + C * 8]
                )

            # ---- tail: transpose z_read back to row layout and store ----
            ncols = T * 8  # = BC
            gsz = min(512, ncols)
            for g0 in range(0, ncols, gsz):
                nk = gsz // 128
                psum_zr = psb.tile([128, gsz], f32, tag="pzr")
                for k in range(nk):
                    j0 = g0 + k * 128
                    nc.tensor.transpose(
                        psum_zr[:, k * 128 : (k + 1) * 128], rt_all[:, j0 : j0 + 128], ident[:]
                    )
                zr_sb = sb.tile([128, gsz], f32, tag="zr")
                nc.vector.tensor_copy(zr_sb[:], psum_zr[:])
                nc.sync.dma_start(
                    out_zr[g0 : g0 + gsz, :].rearrange("(k p) f -> p k f", p=128),
                    zr_sb[:].rearrange("p (k f) -> p k f", f=F),
                )

    nc.compile()
    return nc


def _host_prep(z_real, z_imag, mem, ptr, ctrl, wq_r, wq_i, wk_r, wk_i, wv_r, wv_i):
    """Build per-core input maps. All data transforms are layout-only; the
    only arithmetic is constant-folding of the projection weights."""
    def cmat(wr, wi):
        # x_flat @ W  with  W = [[wr.T, wi.T], [-wi.T, wr.T]]
        w = np.zeros((128, 128), np.float32)
        w[:64, :64] = wr.T
        w[:64, 64:] = wi.T
        w[64:, :64] = -wi.T
        w[64:, 64:] = wr.T
        return w

    Wq = cmat(wq_r, wq_i)
    Wk = cmat(wk_r, wk_i)
    Wv = cmat(wv_r, wv_i)
    A = (Wq @ Wk.T) * np.float32(D ** -0.5)

    consts = {
        "ident_in": np.eye(128, dtype=np.float32),
        "a_in": A.astype(np.float32),
        "wv_in": Wv.astype(np.float32),
        "ones8_in": np.kron(np.eye(8), np.ones((1, 16))).astype(np.float32),
        "mask_in": np.kron(np.eye(8), np.ones((16, 16))).astype(np.float32),
        "g_in": np.kron(np.eye(8), np.ones((16, 1))).astype(np.float32),
    }

    zflat = np.concatenate([z_real, z_imag], axis=1)  # (B, 128)
    in_maps = []
    for core in range(NCORES):
        b0 = core * BC
        memc = np.ascontiguousarray(mem[b0 : b0 + BC].reshape(ROWS, F))
        zc = zflat[b0 : b0 + BC]
        z_wide = np.ascontiguousarray(
            zc.reshape(NQ, 128, F).transpose(1, 0, 2).reshape(128, NQ * F)
        )
        ctrlc = ctrl[b0 : b0 + BC]
        ctrl_wide = np.ascontiguousarray(
            ctrlc.reshape(NQ, 128, 3).transpose(1, 0, 2).reshape(128, NQ * 3)
        )
        # column layout: [bi*16+s, t*3+j] = ctrl[t*8+bi, j]
        cT = np.broadcast_to(
            ctrlc.reshape(T, 8, 1, 3), (T, 8, 16, 3)
        )
        ctrlT = np.ascontiguousarray(cT.transpose(1, 2, 0, 3).reshape(128, T * 3))
        ptrc = ptr[b0 : b0 + BC]

        def pcol(p):
            return np.ascontiguousarray(
                p.reshape(T, 8, 16).transpose(1, 2, 0).reshape(128, T)
            )

        m = {
            "mem_in": memc,
            "z_in": z_wide,
            "ctrlw_in": ctrl_wide,
            "ctrlT_in": ctrlT,
            "ptrv_in": pcol(ptrc),
            "ptru_in": pcol(np.roll(ptrc, 1, axis=1)),
            "ptrd_in": pcol(np.roll(ptrc, -1, axis=1)),
        }
        m.update(consts)
        in_maps.append(m)
    return in_maps


def _gather(results):
    zr_r = np.empty((B, D), np.float32)
    zr_i = np.empty((B, D), np.float32)
    mem_new = np.empty((B, S, F), np.float32)
    new_ptr = np.empty((B, S), np.float32)
    total = 0.0
    for core, r in enumerate(results):
        b0 = core * BC
        zr = r["out_zr"]
        zr_r[b0 : b0 + BC] = zr[:, :D]
        zr_i[b0 : b0 + BC] = zr[:, D:]
        mem_new[b0 : b0 + BC] = r["out_mem"].reshape(BC, S, F)
        # out_nptr[bi*16+s, t] -> new_ptr[t*8+bi, s]
        np_col = r["out_nptr"].reshape(8, 16, T)
        new_ptr[b0 : b0 + BC] = np_col.transpose(2, 0, 1).reshape(BC, S)
        total += float(r["out_cnt"].sum())
    active = np.float32(total / B)
    return zr_r, zr_i, mem_new, new_ptr, active


def kernel(**inputs):
    if "nc" not in _CACHE:
        _CACHE["nc"] = _build_program()
    nc = _CACHE["nc"]
    in_maps = _host_prep(**{k: np.asarray(v) for k, v in inputs.items()})
    res = bass_utils.run_bass_kernel_spmd(nc, in_maps, core_ids=list(range(NCORES)))
    return _gather(res.results)


# revision 10
# speedup vs baseline: 1.2889x; 1.2889x over previous
"""Trainium2 Bass kernel for nn_EnhancedMemoryStack (scatter_memory).

Strategy (pure data parallel, batch sharded 8 ways):
  Per core: 4096 batches. Rows r=(bi,s) b-major, tile = 8 batches = 128 rows.
  - Fold the six 64x64 complex projections into 128x128 real mats on host
    (weight constant-folding): Wq,Wk,Wv; and A = Wq @ Wk.T * D^-0.5 so that
    scores = X A X^T needs ONE projection (Y = X @ A) instead of Q and K.
  - mem_new = mem*(1-push) + push*z; the z broadcast over slots runs on the
    PE (ones8 selector matmul) with the z*push term split hi+lo fp16 so the
    fp32 output stays ~1e-5 accurate at full PE rate.
  - Attention per 128-row tile in fp16/bf16 (PE full rate + fast weight
    load): XT = transpose(mem_new); YT = A.T @ XT; scores = Y X^T (psum,
    8 batches block-diagonal); E = exp(scores); masked rowsum fused in one
    scalar_tensor_tensor; w = E^T (new_ptr/rowsum); the V projection is
    algebraically delayed: u = Gw^T X per tile, z_read = u @ Wv once at the
    end (removes the V matmul, Vw pass and readT transposes entirely).
  - new_ptr/gates computed once per core in a wide column layout; ptr inputs
    are host-permuted (pure layout transforms) so all DMA is contiguous.
"""

import ml_dtypes
import numpy as np

import concourse.bass as bass
import concourse.tile as tile
from concourse import bacc, mybir
from concourse import bass_utils

B, S, D = 32768, 16, 64
F = 2 * D            # 128
NCORES = 8
BC = B // NCORES     # 4096 batches per core
ROWS = BC * S        # 65536 rows per core
T = BC // 8          # 512 tiles per core
C = 8                # tiles per chunk
NCH = T // C         # chunks per core
NQ = BC // 128       # 32 column-groups in wide layouts
EPS = 1e-6

f32 = mybir.dt.float32
f16 = mybir.dt.float16
bf16 = mybir.dt.bfloat16
AF = mybir.ActivationFunctionType
ALU = mybir.AluOpType

_CACHE = {}


def _build_program():
    nc = bacc.Bacc(
        "TRN2",
        target_bir_lowering=False,
        debug=False,
        enable_asserts=False,
        num_devices=NCORES,
    )

    def din(name, shape, dt=f32):
        return nc.dram_tensor(name, list(shape), dt, kind="ExternalInput").ap()

    def dout(name, shape):
        return nc.dram_tensor(name, list(shape), f32, kind="ExternalOutput").ap()

    mem_in = din("mem_in", (ROWS, F))
    z_in = din("z_in", (128, NQ * F))        # z_wide[p, q*F+f] = zflat[q*128+p, f]
    ctrlw_in = din("ctrlw_in", (128, NQ * 3))  # wide: [p, q*3+j] = ctrl[q*128+p, j]
    ctrlT_in = din("ctrlT_in", (128, T * 3))   # col:  [bi*16+s, t*3+j] = ctrl[t*8+bi, j]
    ptrv_in = din("ptrv_in", (128, T))         # col:  [bi*16+s, t] = ptr[t*8+bi, s]
    ptru_in = din("ptru_in", (128, T))         # same, ptr rolled +1 along s
    ptrd_in = din("ptrd_in", (128, T))         # same, ptr rolled -1 along s
    ident_in = din("ident_in", (128, 128), f16)
    identf_in = din("identf_in", (128, 128))
    a_in = din("a_in", (128, 128), f16)
    wv_in = din("wv_in", (128, 128))
    ones8_in = din("ones8_in", (8, 128), f16)  # kron(eye(8), ones(1,16))
    mask_in = din("mask_in", (128, 128), bf16)  # kron(eye(8), ones(16,16))
    grep_in = din("grep_in", (128, C * 8), f16)  # kron(eye(8),ones(16,1)) tiled C

    out_mem = dout("out_mem", (ROWS, F))
    out_nptr = dout("out_nptr", (128, T))
    out_zr = dout("out_zr", (BC, F))
    out_cnt = dout("out_cnt", (128, 1))
    u_dram = nc.dram_tensor("u_scratch", [BC, F], f32, kind="Internal").ap()

    with tile.TileContext(nc) as tc:
        with (
            tc.tile_pool(name="const", bufs=1) as cpool,
            tc.tile_pool(name="sb", bufs=3) as sb,
            tc.tile_pool(name="sb2", bufs=3) as sb2,
            tc.tile_pool(name="ps", bufs=2, space=bass.MemorySpace.PSUM) as ps,
            tc.tile_pool(name="psb", bufs=1, space=bass.MemorySpace.PSUM) as psb,
        ):
            # ---- resident constants ----
            ident = cpool.tile([128, 128], f16)
            nc.sync.dma_start(ident[:], ident_in)
            identf = cpool.tile([128, 128], f32)
            nc.sync.dma_start(identf[:], identf_in)
            a_t = cpool.tile([128, 128], f16)
            nc.sync.dma_start(a_t[:], a_in)
            wv_t = cpool.tile([128, 128], f32)
            nc.sync.dma_start(wv_t[:], wv_in)
            ones8 = cpool.tile([8, 128], f16)
            nc.sync.dma_start(ones8[:], ones8_in)
            mask_t = cpool.tile([128, 128], bf16)
            nc.sync.dma_start(mask_t[:], mask_in)
            grep = cpool.tile([128, C * 8], f16)
            nc.sync.dma_start(grep[:], grep_in)

            ctrlT = cpool.tile([128, T * 3], f32)
            nc.sync.dma_start(ctrlT[:], ctrlT_in)
            ptrv = cpool.tile([128, T], f32)
            nc.sync.dma_start(ptrv[:], ptrv_in)
            ptru = cpool.tile([128, T], f32)
            nc.sync.dma_start(ptru[:], ptru_in)
            ptrd = cpool.tile([128, T], f32)
            nc.sync.dma_start(ptrd[:], ptrd_in)
            zw = cpool.tile([128, NQ * F], f32)
            nc.sync.dma_start(zw[:], z_in)
            ctrlw = cpool.tile([128, NQ * 3], f32)
            nc.sync.dma_start(ctrlw[:], ctrlw_in)

            # ---- gates in column layout (once per core) ----
            g3 = cpool.tile([128, T * 3], f32)
            nc.scalar.activation(g3[:], ctrlT[:], AF.Sigmoid)
            g3v = g3[:].rearrange("p (t j) -> p t j", j=3)
            tot = cpool.tile([128, T], f32)
            nc.vector.tensor_reduce(tot[:], g3v, axis=mybir.AxisListType.X, op=ALU.add)
            nc.vector.tensor_scalar_add(tot[:], tot[:], EPS)
            rec = cpool.tile([128, T], f32)
            nc.vector.reciprocal(rec[:], tot[:])
            push = cpool.tile([128, T], f32)
            nc.vector.tensor_mul(push[:], g3v[:, :, 0], rec[:])
            omp = cpool.tile([128, T], f32)   # 1 - push
            nc.vector.tensor_scalar(omp[:], push[:], -1.0, 1.0, op0=ALU.mult, op1=ALU.add)
            npt = cpool.tile([128, T], f32)   # new_ptr (column layout)
            tmp = cpool.tile([128, T], f32)
            nc.vector.tensor_mul(npt[:], push[:], ptru[:])
            nc.vector.tensor_mul(tmp[:], g3v[:, :, 1], rec[:])       # pop
            nc.vector.tensor_mul(tmp[:], tmp[:], ptrd[:])
            nc.vector.tensor_add(npt[:], npt[:], tmp[:])
            nc.vector.tensor_mul(tmp[:], g3v[:, :, 2], rec[:])       # stay
            nc.vector.tensor_mul(tmp[:], tmp[:], ptrv[:])
            nc.vector.tensor_add(npt[:], npt[:], tmp[:])
            nc.sync.dma_start(out_nptr, npt[:])
            gt = cpool.tile([128, T], f32)
            nc.vector.tensor_scalar(gt[:], npt[:], 0.1, None, op0=ALU.is_gt)
            cnt = cpool.tile([128, 1], f32)
            nc.vector.tensor_reduce(cnt[:], gt[:], axis=mybir.AxisListType.X, op=ALU.add)
            nc.sync.dma_start(out_cnt, cnt[:])

            # ---- zp = z * push (wide layout), split into fp16 hi + lo ----
            g3w = cpool.tile([128, NQ * 3], f32)
            nc.scalar.activation(g3w[:], ctrlw[:], AF.Sigmoid)
            g3wv = g3w[:].rearrange("p (q j) -> p q j", j=3)
            totw = cpool.tile([128, NQ], f32)
            nc.vector.tensor_reduce(totw[:], g3wv, axis=mybir.AxisListType.X, op=ALU.add)
            nc.vector.tensor_scalar_add(totw[:], totw[:], EPS)
            recw = cpool.tile([128, NQ], f32)
            nc.vector.reciprocal(recw[:], totw[:])
            pushw = cpool.tile([128, NQ], f32)
            nc.vector.tensor_mul(pushw[:], g3wv[:, :, 0], recw[:])
            zp = cpool.tile([128, NQ * F], f32)
            pushw_b = pushw[:].unsqueeze(-1).broadcast_to([128, NQ, F])
            nc.vector.tensor_mul(
                zp[:].rearrange("p (q f) -> p q f", f=F),
                zw[:].rearrange("p (q f) -> p q f", f=F),
                pushw_b,
            )
            zph = cpool.tile([128, NQ * F], f16)
            nc.scalar.activation(zph[:], zp[:], AF.Copy)
            zpl = cpool.tile([128, NQ * F], f16)
            nc.vector.tensor_sub(zpl[:], zp[:], zph[:])

            CF = C * F

            # ---- main loop over chunks ----
            for ch in range(NCH):
                t0 = ch * C
                r0 = t0 * 128

                mem_c = sb.tile([128, CF], f32, tag="mem")
                nc.sync.dma_start(
                    mem_c[:].rearrange("p (c f) -> p c f", f=F),
                    mem_in[r0 : r0 + C * 128, :].rearrange("(c p) f -> p c f", p=128),
                )
                # stage z*push (hi/lo fp16) slices into partitions 0..7
                zph8 = sb.tile([8, CF], f16, tag="zph8")
                zpl8 = sb.tile([8, CF], f16, tag="zpl8")
                for c in range(C):
                    po = ((t0 + c) * 8) % 128
                    q = ((t0 + c) * 8) // 128
                    nc.sync.dma_start(
                        zph8[:, c * F : (c + 1) * F], zph[po : po + 8, q * F : (q + 1) * F]
                    )
                    nc.sync.dma_start(
                        zpl8[:, c * F : (c + 1) * F], zpl[po : po + 8, q * F : (q + 1) * F]
                    )
                psum_z = ps.tile([128, CF], f32, tag="big")
                for h in range(C // 4):  # N=512 per matmul (psum bank limit)
                    hs = slice(h * 512, (h + 1) * 512)
                    nc.tensor.matmul(psum_z[:, hs], ones8[:], zph8[:, hs], start=True, stop=False)
                    nc.tensor.matmul(psum_z[:, hs], ones8[:], zpl8[:, hs], start=False, stop=True)
                # mem_new = mem*(1-push) + psum_z   (fused per tile)
                mnew = sb.tile([128, CF], f32, tag="mnew")
                for c in range(C):
                    cs = slice(c * F, (c + 1) * F)
                    nc.vector.scalar_tensor_tensor(
                        mnew[:, cs], mem_c[:, cs], omp[:, t0 + c : t0 + c + 1],
                        psum_z[:, cs], op0=ALU.mult, op1=ALU.add,
                    )
                nc.sync.dma_start(
                    out_mem[r0 : r0 + C * 128, :].rearrange("(c p) f -> p c f", p=128),
                    mnew[:].rearrange("p (c f) -> p c f", f=F),
                )
                mnb = sb2.tile([128, CF], f16, tag="mnb")
                nc.scalar.activation(mnb[:], mnew[:], AF.Copy)
                psum_xt = ps.tile([128, CF], f16, tag="big")
                for c in range(C):
                    cs = slice(c * F, (c + 1) * F)
                    nc.tensor.transpose(psum_xt[:, cs], mnb[:, cs], ident[:])
                xt = sb2.tile([128, CF], f16, tag="xt")
                nc.scalar.activation(xt[:], psum_xt[:], AF.Copy)
                psum_yt = ps.tile([128, CF], f32, tag="big")
                for h in range(C // 4):
                    hs = slice(h * 512, (h + 1) * 512)
                    nc.tensor.matmul(psum_yt[:, hs], a_t[:], xt[:, hs], start=True, stop=True)
                yt = sb2.tile([128, CF], f16, tag="yt")
                nc.scalar.activation(yt[:], psum_yt[:], AF.Copy)
                psum_sc = ps.tile([128, CF], f32, tag="big")
                for c in range(C):
                    cs = slice(c * F, (c + 1) * F)
                    nc.tensor.matmul(psum_sc[:, cs], yt[:, cs], xt[:, cs], start=True, stop=True)
                eraw = sb2.tile([128, CF], bf16, tag="eraw")
                nc.scalar.activation(eraw[:], psum_sc[:], AF.Exp)
                e_t = sb2.tile([128, CF], bf16, tag="e")
                rs = sb.tile([128, C], f32, tag="rs")
                for c in range(C):
                    cs = slice(c * F, (c + 1) * F)
                    nc.vector.scalar_tensor_tensor(
                        e_t[:, cs], eraw[:, cs], 1.0, mask_t[:],
                        op0=ALU.mult, op1=ALU.mult,
                        accum_out=rs[:, c : c + 1],
                    )
                rr = sb.tile([128, C], f32, tag="rr")
                nc.vector.reciprocal(rr[:], rs[:])
                ptrn = sb.tile([128, C], bf16, tag="ptrn")
                nc.vector.tensor_mul(ptrn[:], npt[:, t0 : t0 + C], rr[:])
                psum_w = psb.tile([128, C], f32, tag="pw")
                for c in range(C):
                    cs = slice(c * F, (c + 1) * F)
                    nc.tensor.matmul(
                        psum_w[:, c : c + 1], e_t[:, cs], ptrn[:, c : c + 1],
                        start=True, stop=True,
                    )
                gw = sb.tile([128, C * 8], f16, tag="gw")
                w_b = psum_w[:].unsqueeze(-1).broadcast_to([128, C, 8])
                nc.vector.tensor_mul(
                    gw[:].rearrange("p (c g) -> p c g", g=8), grep[:].rearrange("p (c g) -> p c g", g=8), w_b
                )
                psum_u = psb.tile([8, CF], f32, tag="pu")
                for c in range(C):
                    cs = slice(c * F, (c + 1) * F)
                    nc.tensor.matmul(
                        psum_u[:, cs], gw[:, c * 8 : (c + 1) * 8], mnb[:, cs],
                        start=True, stop=True,
                    )
                u_sb = sb.tile([8, CF], f32, tag="usb")
                nc.vector.tensor_copy(u_sb[:], psum_u[:])
                nc.sync.dma_start(
                    u_dram[ch * C * 8 : (ch + 1) * C * 8, :].rearrange("(c b) f -> b c f", b=8),
                    u_sb[:].rearrange("b (c f) -> b c f", f=F),
                )

            # ---- tail: z_read = u @ Wv, back in row layout ----
            gsz = min(512, BC)
            for g0 in range(0, BC, gsz):
                nk = gsz // 128
                ug = sb.tile([128, gsz], f32, tag="ug")
                nc.sync.dma_start(
                    ug[:].rearrange("p (k f) -> p k f", f=F),
                    u_dram[g0 : g0 + gsz, :].rearrange("(k p) f -> p k f", p=128),
                )
                psum_ut = ps.tile([128, gsz], f32, tag="big")
                for k in range(nk):
                    ks = slice(k * 128, (k + 1) * 128)
                    nc.tensor.transpose(psum_ut[:, ks], ug[:, ks], identf[:])
                utb = sb.tile([128, gsz], f32, tag="utb")
                nc.vector.tensor_copy(utb[:], psum_ut[:])
                psum_zr = ps.tile([128, gsz], f32, tag="big")
                for k in range(nk):
                    ks = slice(k * 128, (k + 1) * 128)
                    nc.tensor.matmul(psum_zr[:, ks], utb[:, ks], wv_t[:], start=True, stop=True)
                zr_sb = sb.tile([128, gsz], f32, tag="zr")
                nc.vector.tensor_copy(zr_sb[:], psum_zr[:])
                nc.sync.dma_start(
                    out_zr[g0 : g0 + gsz, :].rearrange("(k p) f -> p k f", p=128),
                    zr_sb[:].rearrange("p (k f) -> p k f", f=F),
                )

    nc.compile()
    return nc


def _host_prep(z_real, z_imag, mem, ptr, ctrl, wq_r, wq_i, wk_r, wk_i, wv_r, wv_i):
    """Build per-core input maps. All data transforms are layout-only; the
    only arithmetic is constant-folding of the projection weights."""
    def cmat(wr, wi):
        # x_flat @ W  with  W = [[wr.T, wi.T], [-wi.T, wr.T]]
        w = np.zeros((128, 128), np.float32)
        w[:64, :64] = wr.T
        w[:64, 64:] = wi.T
        w[64:, :64] = -wi.T
        w[64:, 64:] = wr.T
        return w

    Wq = cmat(wq_r, wq_i)
    Wk = cmat(wk_r, wk_i)
    Wv = cmat(wv_r, wv_i)
    A = (Wq @ Wk.T) * np.float32(D ** -0.5)

    consts = {
        "ident_in": np.eye(128, dtype=np.float16),
        "identf_in": np.eye(128, dtype=np.float32),
        "a_in": A.astype(np.float16),
        "wv_in": Wv.astype(np.float32),
        "ones8_in": np.kron(np.eye(8), np.ones((1, 16))).astype(np.float16),
        "mask_in": np.kron(np.eye(8), np.ones((16, 16))).astype(ml_dtypes.bfloat16),
        "grep_in": np.tile(np.kron(np.eye(8), np.ones((16, 1))), (1, C)).astype(np.float16),
    }

    zflat = np.concatenate([z_real, z_imag], axis=1)  # (B, 128)
    in_maps = []
    for core in range(NCORES):
        b0 = core * BC
        memc = np.ascontiguousarray(mem[b0 : b0 + BC].reshape(ROWS, F))
        zc = zflat[b0 : b0 + BC]
        z_wide = np.ascontiguousarray(
            zc.reshape(NQ, 128, F).transpose(1, 0, 2).reshape(128, NQ * F)
        )
        ctrlc = ctrl[b0 : b0 + BC]
        ctrl_wide = np.ascontiguousarray(
            ctrlc.reshape(NQ, 128, 3).transpose(1, 0, 2).reshape(128, NQ * 3)
        )
        cT = np.broadcast_to(ctrlc.reshape(T, 8, 1, 3), (T, 8, 16, 3))
        ctrlT = np.ascontiguousarray(cT.transpose(1, 2, 0, 3).reshape(128, T * 3))
        ptrc = ptr[b0 : b0 + BC]

        def pcol(p):
            return np.ascontiguousarray(
                p.reshape(T, 8, 16).transpose(1, 2, 0).reshape(128, T)
            )

        m = {
            "mem_in": memc,
            "z_in": z_wide,
            "ctrlw_in": ctrl_wide,
            "ctrlT_in": ctrlT,
            "ptrv_in": pcol(ptrc),
            "ptru_in": pcol(np.roll(ptrc, 1, axis=1)),
            "ptrd_in": pcol(np.roll(ptrc, -1, axis=1)),
        }
        m.update(consts)
        in_maps.append(m)
    return in_maps


def _gather(results):
    zr_r = np.empty((B, D), np.float32)
    zr_i = np.empty((B, D), np.float32)
    mem_new = np.empty((B, S, F), np.float32)
    new_ptr = np.empty((B, S), np.float32)
    total = 0.0
    for core, r in enumerate(results):
        b0 = core * BC
        zr = r["out_zr"]
        zr_r[b0 : b0 + BC] = zr[:, :D]
        zr_i[b0 : b0 + BC] = zr[:, D:]
        mem_new[b0 : b0 + BC] = r["out_mem"].reshape(BC, S, F)
        np_col = r["out_nptr"].reshape(8, 16, T)
        new_ptr[b0 : b0 + BC] = np_col.transpose(2, 0, 1).reshape(BC, S)
        total += float(r["out_cnt"].sum())
    active = np.float32(total / B)
    return zr_r, zr_i, mem_new, new_ptr, active


def kernel(**inputs):
    if "nc" not in _CACHE:
        _CACHE["nc"] = _build_program()
    nc = _CACHE["nc"]
    in_maps = _host_prep(**{k: np.asarray(v) for k, v in inputs.items()})
    res = bass_utils.run_bass_kernel_spmd(nc, in_maps, core_ids=list(range(NCORES)))
    return _gather(res.results)
